# revision 22
# baseline (speedup 1.0000x reference)
"""Trainium2 Bass kernel for PVT-style spatial-reduction multi-head attention.

Problem (hardcoded shapes, fp32 inputs):
  x [2, 4096, 512]; Wq [512,512]; Wconv [512,512,2,2] (OIHW, stride 2);
  LayerNorm over the conv's flattened spatial dim (M=1024); Wkv [1024,1024];
  attention with q [B,8,4096,64], k/v [B,8,512,64]; "faithful" reshape
  (out.transpose(0,1,3,2).reshape(B,-1,512)) before Wproj [512,512].

Sharding: 8 cores = (batch b in {0,1}) x (head-pair g in {0..3}).
Core (b,g) computes heads {2g, 2g+1} of batch b and writes output rows
[b, 1024g : 1024g+1024, :].

v3 design vs the v2 baseline (113 us):
 - x is sent host-side in a tap-expanded transposed layout Xp[ic, (m, tap)]
   (n = 128i + 64di + 2j + dj; m = 32i + j; tap = 2di + dj), so there is no
   on-chip x transpose. Q and the conv consume Xp directly; the resulting
   within-chunk column permutation of q (and of the attention output) is
   absorbed into a host-side row permutation of Wproj.
 - The stride-2 2x2 VALID conv is non-overlapping and is computed
   TRANSPOSED (xcT [m, o] = Xp^T @ Wconv'), which feeds the KV matmul with
   no transpose stage. (A 4-way m-sharded variant with a DRAM AllGather was
   tried and reverted: one collective_compute costs ~150-250 us through
   this NRT path, dwarfing the 20 us of saved conv work.)
 - LayerNorm is folded algebraically: gamma into Wkv rows (host), beta+bkv
   into a bias row s (host), so kv_noscale = xcT @ Wkv' - mu_c t + s*sqrtve_c
   with the mu/s terms as K=1 rank-1 PE matmuls into the same PSUM group.
   The per-position scale rs_c = rsqrt(var_c+eps) folds into the softmax
   exp scale (k side, per-partition AP scale) and the vaug scale (v side).
   Stats (sum x, sum x^2) come from ones-vector PE matmuls; rsqrt is
   computed as exp(-0.5*ln(v)) so the ACT engine stays on the single
   activation table holding exp/ln/copy/square/identity (no 1283ns act
   table reloads anywhere in the kernel).
 - AV is computed n-partitioned (av2[n, d] = sum_c phat[c,n]^T [v^T*rs|1]),
   so there is no AV transpose stage, and softmax denominators ride along
   as a 65th rhs column, landing pre-transposed for the normalize.
 - Engine balance: ACT does exp + PSUM->SBUF copies (one act table), DVE
   does Q epilogue, squares, AV normalize; reciprocal+normalize are
   interleaved into the chunk loop so only the projection remains as tail.
 - KV is computed in four c-column groups landing in separate kT4/vT4
   tiles (one writer each), so the first S matmul and the vaug transposes
   start as soon as their group finishes instead of waiting for the full
   512-column kT/vT accumulation.

Known dead end (do not retry naively): pairing two chunks into one
[128,1024] PSUM tile (one exp for two S^T tiles) and, separately, two exps
writing halves of one phat tile, both produced sporadic inf corruption on
one core — two WRITER instructions filling halves of a single tile race
with a reader of the second half under this Tile version's dependency
tracking. Keep one writer per tile, or add explicit ordering, if
reattempting the exp-pairing (~4us ACT win) or in-loop projection overlap
(~6us tail win).
"""

import sys

sys.path.insert(0, "/opt/trn_rl_repo")

import math

import numpy as np
import ml_dtypes

import concourse.bass as bass
import concourse.bacc as bacc
import concourse.mybir as mybir
import concourse.tile as tile
from concourse.bass_utils import run_bass_kernel_spmd

F32 = mybir.dt.float32
BF16 = mybir.dt.bfloat16
NP_BF16 = ml_dtypes.bfloat16

B, N, C = 2, 4096, 512
NH, HD, SR = 8, 64, 2
M = 1024
EPS = 1e-5
N_CORES = 8

# rows16 row map ([8, 512] bf16 host constants)
ROW_BCONV = 0
ROW_BPROJ = 1
ROW_TK = 2
ROW_TV = 3
ROW_SK = 4
ROW_SV = 5
ROW_ONES = 6


def _build_module(reps=1, bench_internal=False):
    nc = bacc.Bacc("TRN2", target_bir_lowering=False, debug=False)

    # ---- per-core DRAM tensors (data differs per core, shapes identical) ----
    KIND = "Internal" if bench_internal else "ExternalInput"
    xp = nc.dram_tensor("xp", [C, N], BF16, kind=KIND).ap()
    wq = nc.dram_tensor("wq", [C, 128], BF16, kind=KIND).ap()
    wconvt = nc.dram_tensor("wconvt", [C, 4, C], BF16, kind=KIND).ap()
    wkv2 = nc.dram_tensor("wkv2", [M, 256], BF16, kind=KIND).ap()
    wproj = nc.dram_tensor("wproj", [C, C], BF16, kind=KIND).ap()
    small = nc.dram_tensor("small", [128, 2], F32, kind="ExternalInput").ap()
    rows16 = nc.dram_tensor("rows16", [1, 8 * C], BF16, kind=KIND).ap()
    eye128 = nc.dram_tensor("eye128", [128, 128], BF16, kind=KIND).ap()
    eyef = nc.dram_tensor("eyef", [8, 8], F32, kind=KIND).ap()
    OKIND = "Internal" if bench_internal else "ExternalOutput"
    out = nc.dram_tensor("out", [2 * C, C], F32, kind=OKIND).ap()
    dummy = (
        nc.dram_tensor("bench_out", [1, 2], F32, kind="ExternalOutput").ap()
        if bench_internal
        else None
    )

    AX = mybir.AxisListType.X
    OP = mybir.AluOpType
    AF = mybir.ActivationFunctionType
    LN8 = math.log(0.125)

    with tile.TileContext(nc) as tc:
        import contextlib

        with contextlib.ExitStack() as ctx:
            persist = ctx.enter_context(tc.tile_pool(name="persist", bufs=1))
            stage = ctx.enter_context(tc.tile_pool(name="stage", bufs=3))
            ps512 = ctx.enter_context(tc.tile_pool(name="ps512", bufs=3, space="PSUM"))
            psa2 = ctx.enter_context(tc.tile_pool(name="psa2", bufs=3, space="PSUM"))
            pstp = ctx.enter_context(tc.tile_pool(name="pstp", bufs=2, space="PSUM"))

            for _rep in range(reps):
                # ---------------- weight / const loads ----------------
                wq_sb = []
                for k in range(4):
                    t = persist.tile([128, 128], BF16, name=f"wq_sb{k}", tag=f"wq{k}")
                    nc.sync.dma_start(t[:], wq[128 * k : 128 * (k + 1), :])
                    wq_sb.append(t)

                wconv_sb = []  # [ic_t] -> [128 ic, (tap 4, o 512)]
                for kt in range(4):
                    t = persist.tile([128, 2048], BF16, name=f"wconv{kt}", tag=f"wc{kt}")
                    nc.sync.dma_start(t[:], wconvt[128 * kt : 128 * (kt + 1), :, :])
                    wconv_sb.append(t)

                wkv_sb = []  # [mt] -> [128 m, 256] (cols 0:128 k, 128:256 v)
                for k in range(8):
                    t = persist.tile([128, 256], BF16, name=f"wkv_sb{k}", tag=f"wkv{k}")
                    nc.sync.dma_start(t[:], wkv2[128 * k : 128 * (k + 1), :])
                    wkv_sb.append(t)

                wproj_sb = []
                for ct in range(4):
                    t = persist.tile([128, 512], BF16, name=f"wproj{ct}", tag=f"wp{ct}")
                    nc.sync.dma_start(t[:], wproj[128 * ct : 128 * (ct + 1), :])
                    wproj_sb.append(t)

                small_sb = persist.tile([128, 2], F32, name="small_sb", tag="small")
                nc.sync.dma_start(small_sb[:], small[:, :])
                if dummy is not None and _rep == 0:
                    nc.sync.dma_start(dummy[:, :], small[0:1, 0:2])
                rows_sb = persist.tile([1, 4096], BF16, name="rows_sb", tag="rows16")
                nc.sync.dma_start(rows_sb[:], rows16[:, :])

                def crow(r, n=512):
                    return rows_sb[0:1, 512 * r : 512 * r + n]
                eye_sb = persist.tile([128, 128], BF16, name="eye_sb", tag="eye128")
                nc.sync.dma_start(eye_sb[:], eye128[:, :])
                eyef_sb = persist.tile([8, 8], F32, name="eyef_sb", tag="eyef")
                nc.sync.dma_start(eyef_sb[:], eyef[:, :])
                onec_sb = persist.tile([128, 1], BF16, name="onec_sb", tag="onec")
                nc.vector.memset(onec_sb[:], 1.0)
                ln8_sb = persist.tile([128, 1], F32, name="ln8_sb", tag="ln8")
                nc.vector.memset(ln8_sb[:], LN8)

                bq_col = small_sb[:, 0:1]

                # ---------------- x load (conv slices first) ----------------
                xp_sb = []  # [ic_t] -> [128 ic, 4096 (m,tap)]
                for kt in range(4):
                    t = persist.tile([128, N], BF16, name=f"xp_sb{kt}", tag=f"xp{kt}")
                    xp_sb.append(t)
                for kt in range(4):
                    nc.sync.dma_start(
                        xp_sb[kt][:, 0:1024], xp[128 * kt : 128 * (kt + 1), 0:1024]
                    )
                for kt in range(4):
                    nc.sync.dma_start(
                        xp_sb[kt][:, 1024:4096], xp[128 * kt : 128 * (kt + 1), 1024:4096]
                    )
                xp4 = [t.rearrange("p (m tap) -> p m tap", m=M, tap=4) for t in xp_sb]

                # ---------------- conv (all 8 m-blocks), stats ----------------
                xcl_sb = []  # conv out [128 m, 512 o] bf16 per m-block
                sq_sb = []
                for l in range(8):
                    c_ps = ps512.tile([128, 512], F32, name="c_ps", tag="mm512")
                    first = True
                    # bconv is NOT added here: LayerNorm variance is
                    # shift-invariant and the mu*t rank-1 in KV cancels the
                    # t*bconv term exactly, so the bias-free conv output
                    # with bias-free stats yields identical kv/attention
                    # results for ANY bconv (verified to 1e-13 in numpy).
                    for kt in range(4):
                        for tap in range(4):
                            nc.tensor.matmul(
                                c_ps[:],
                                xp4[kt][:, 128 * l : 128 * (l + 1), tap],
                                wconv_sb[kt][:, 512 * tap : 512 * (tap + 1)],
                                start=first,
                                stop=(kt == 3 and tap == 3),
                            )
                            first = False
                    xcl = persist.tile([128, 512], BF16, name=f"xcl{l}", tag=f"xcl{l}")
                    nc.scalar.activation(xcl[:], c_ps[:], AF.Copy)
                    sq = persist.tile([128, 512], BF16, name=f"sq{l}", tag=f"sql{l}")
                    nc.vector.tensor_mul(sq[:], xcl[:], xcl[:])
                    xcl_sb.append(xcl)
                    sq_sb.append(sq)
                sx_ps = ps512.tile([1, 512], F32, name="sx_ps", tag="mm512")
                sq_ps = ps512.tile([1, 512], F32, name="sq_ps", tag="mm512")
                for l in range(8):
                    nc.tensor.matmul(
                        sx_ps[:], onec_sb[:], xcl_sb[l][:],
                        start=(l == 0), stop=(l == 7),
                    )
                    nc.tensor.matmul(
                        sq_ps[:], onec_sb[:], sq_sb[l][:],
                        start=(l == 0), stop=(l == 7),
                    )
                srow_sb = persist.tile([1, 1024], F32, name="srow_sb", tag="srow")
                nc.vector.tensor_copy(srow_sb[0:1, 0:512], sx_ps[:])
                nc.vector.tensor_copy(srow_sb[0:1, 512:1024], sq_ps[:])
                sx_row = srow_sb[0:1, 0:512]
                sq_row = srow_sb[0:1, 512:1024]

                # ---------------- Q projection ----------------
                qt_sb = persist.tile([128, N], BF16, name="qt_sb", tag="qt")
                for ch in range(8):
                    q_ps = ps512.tile([128, 512], F32, name="q_ps", tag="mm512")
                    for k in range(4):
                        nc.tensor.matmul(
                            q_ps[:],
                            wq_sb[k][:],
                            xp4[k][:, 128 * ch : 128 * (ch + 1), :],
                            start=(k == 0),
                            stop=(k == 3),
                        )
                    nc.vector.tensor_scalar_add(
                        qt_sb[:, 512 * ch : 512 * (ch + 1)], q_ps[:], bq_col
                    )

                # ---------------- stats math ----------------
                murow = stage.tile([1, 512], F32, name="murow", tag="murow", bufs=1)
                nc.vector.tensor_scalar_mul(murow[:], sx_row, 1.0 / M)
                negmu16 = persist.tile([1, 512], BF16, name="negmu16", tag="negmu")
                nc.vector.tensor_scalar_mul(negmu16[:], sx_row, -1.0 / M)
                verow = persist.tile([1, 512], F32, name="verow", tag="verow")
                nc.vector.tensor_scalar(
                    out=verow[:], in0=sq_row,
                    scalar1=1.0 / M, scalar2=EPS, op0=OP.mult, op1=OP.add,
                )
                mu2 = stage.tile([1, 512], F32, name="mu2", tag="mu2", bufs=1)
                nc.vector.tensor_mul(mu2[:], murow[:], murow[:])
                nc.vector.tensor_sub(verow[:], verow[:], mu2[:])
                # sqrtve row (bf16) = exp(0.5 ln ve)
                lrow = stage.tile([1, 512], F32, name="lrow", tag="lrow", bufs=1)
                nc.scalar.activation(lrow[:], verow[:], AF.Ln)
                sqve16 = persist.tile([1, 512], BF16, name="sqve16", tag="sqve")
                nc.scalar.activation(sqve16[:], lrow[:], AF.Exp, scale=0.5)
                # columns: ve -> [128, 4] via PE transpose, then exp/ln scales
                vecol_ps = ps512.tile([128, 4], F32, name="vecol_ps", tag="mm512")
                for j in range(4):
                    nc.tensor.transpose(
                        vecol_ps[:, j : j + 1],
                        verow[:, 128 * j : 128 * (j + 1)],
                        eyef_sb[0:1, 0:1],
                    )
                lcol = stage.tile([128, 4], F32, name="lcol", tag="lcol", bufs=1)
                nc.scalar.activation(lcol[:], vecol_ps[:], AF.Ln)
                esc_col = persist.tile([128, 4], F32, name="esc_col", tag="esc")
                nc.scalar.activation(esc_col[:], lcol[:], AF.Exp, scale=-0.5, bias=ln8_sb[:])
                vsc_col = persist.tile([128, 4], F32, name="vsc_col", tag="vsc")
                nc.scalar.activation(vsc_col[:], lcol[:], AF.Exp, scale=-0.5)

                # ---------------- KV ----------------
                # split by c-column group: each [128,128] group lands in its
                # own tile so the first S / vaug work starts as soon as its
                # group finishes (single writer per tile).
                kT4 = [
                    persist.tile([128, 128], BF16, name=f"kT4_{mt}", tag=f"kT{mt}")
                    for mt in range(4)
                ]
                vT4 = [
                    persist.tile([128, 128], BF16, name=f"vT4_{mt}", tag=f"vT{mt}")
                    for mt in range(4)
                ]
                for which, lo, t_row, s_row, dst4 in (
                    ("k", 0, ROW_TK, ROW_SK, kT4),
                    ("v", 128, ROW_TV, ROW_SV, vT4),
                ):
                    for g4 in range(4):
                        kv_ps = ps512.tile([128, 128], F32, name="kv_ps", tag="mm512")
                        for k in range(8):
                            nc.tensor.matmul(
                                kv_ps[:],
                                wkv_sb[k][:, lo : lo + 128],
                                xcl_sb[k][:, 128 * g4 : 128 * (g4 + 1)],
                                start=(k == 0),
                                stop=False,
                            )
                        nc.tensor.matmul(
                            kv_ps[:], crow(t_row, 128),
                            negmu16[0:1, 128 * g4 : 128 * (g4 + 1)],
                            start=False, stop=False,
                        )
                        nc.tensor.matmul(
                            kv_ps[:], crow(s_row, 128),
                            sqve16[0:1, 128 * g4 : 128 * (g4 + 1)],
                            start=False, stop=True,
                        )
                        nc.scalar.activation(dst4[g4][:], kv_ps[:], AF.Copy)

                # vaug[p][mt]: [128 c, 64 d] bf16, v^T with rs folded
                vaug_sb = []
                for p in range(2):
                    row = []
                    for mt in range(4):
                        t = persist.tile(
                            [128, 65], BF16, name=f"vaug{p}_{mt}", tag=f"va{p}{mt}"
                        )
                        tp = pstp.tile([128, 64], BF16, name="tpv", tag="ptp")
                        nc.tensor.transpose(
                            tp[:],
                            vT4[mt][64 * p : 64 * (p + 1), :],
                            eye_sb[64 * p : 64 * (p + 1), 64 * p : 64 * (p + 1)],
                        )
                        nc.vector.tensor_scalar_mul(
                            t[:, 0:64], tp[:], vsc_col[:, mt : mt + 1]
                        )
                        nc.vector.memset(t[:, 64:65], 1.0)
                        row.append(t)
                    vaug_sb.append(row)

                # ---------------- attention ----------------
                out2dT = []
                for p in range(2):
                    o_row = []
                    for ct in range(4):
                        t = persist.tile(
                            [128, 512], BF16, name=f"o2dT{p}_{ct}", tag=f"o2{p}{ct}"
                        )
                        o_row.append(t)
                    out2dT.append(o_row)

                def emit_s(ch, phat_all):
                    phat = {0: [], 1: []}
                    for mt in range(4):
                        for p in range(2):
                            s_ps = ps512.tile([128, 512], F32, name="s_ps", tag="mm512")
                            nc.tensor.matmul(
                                s_ps[:],
                                kT4[mt][64 * p : 64 * (p + 1), :],
                                qt_sb[64 * p : 64 * (p + 1), 512 * ch : 512 * (ch + 1)],
                                start=True,
                                stop=True,
                            )
                            ph = stage.tile(
                                [128, 512], BF16, name="phat", tag="phat", bufs=16
                            )
                            nc.scalar.activation(
                                ph[:], s_ps[:], AF.Exp, scale=esc_col[:, mt : mt + 1]
                            )
                            phat[p].append(ph)
                    phat_all[ch] = phat

                def emit_av(ch, phat_all):
                    phat = phat_all.pop(ch)
                    for p in range(2):
                        av2 = psa2.tile([128, 260], F32, name="av2", tag="psa2")
                        av3 = av2.rearrange("p (ct u) -> p ct u", ct=4, u=65)
                        for ct in range(4):
                            for mt in range(4):
                                nc.tensor.matmul(
                                    av3[:, ct, :],
                                    phat[p][mt][:, 128 * ct : 128 * (ct + 1)],
                                    vaug_sb[p][mt][:],
                                    start=(mt == 0),
                                    stop=(mt == 3),
                                )
                        rc = stage.tile([128, 4], F32, name="rc", tag="rc", bufs=4)
                        nc.vector.reciprocal(rc[:], av3[:, :, 64])
                        for ct in range(4):
                            o3 = out2dT[p][ct].rearrange(
                                "p (d s) -> p d s", d=64, s=8
                            )
                            nc.vector.tensor_scalar_mul(
                                o3[:, :, ch], av3[:, ct, 0:64], rc[:, ct : ct + 1]
                            )

                # software pipeline: S/exp of chunk ch+1 is emitted before
                # AV of chunk ch so the in-order PE never sits on an AV
                # matmul waiting for exp to drain.
                phat_all = {}
                emit_s(0, phat_all)
                for ch in range(8):
                    if ch + 1 < 8:
                        emit_s(ch + 1, phat_all)
                    emit_av(ch, phat_all)

                # ---------------- projection + output ----------------
                for p in range(2):
                    for rt in range(4):
                        pr_ps = ps512.tile([128, 512], F32, name="pr_ps", tag="mm512")
                        for ct in range(4):
                            nc.tensor.matmul(
                                pr_ps[:],
                                out2dT[p][ct][:, 128 * rt : 128 * (rt + 1)],
                                wproj_sb[ct][:],
                                start=(ct == 0),
                                stop=False,
                            )
                        nc.tensor.matmul(
                            pr_ps[:], crow(ROW_ONES, 128), crow(ROW_BPROJ),
                            start=False, stop=True,
                        )
                        of = stage.tile([128, 512], F32, name="of", tag="of", bufs=4)
                        if rt % 2 == 0:
                            nc.scalar.activation(of[:], pr_ps[:], AF.Copy)
                        else:
                            nc.vector.tensor_copy(of[:], pr_ps[:])
                        r0 = 512 * p + 128 * rt
                        nc.sync.dma_start(out[r0 : r0 + 128, :], of[:])

    nc.compile()
    return nc


_NC_CACHE = None


def _get_module():
    global _NC_CACHE
    if _NC_CACHE is None:
        _NC_CACHE = _build_module()
    return _NC_CACHE


def _prep_core_inputs(inputs):
    """Host-side sharding: layout/permute/cast weights, build 8 in_maps."""
    x = np.asarray(inputs["x"], np.float32)
    Wq = np.asarray(inputs["Wq"], np.float32)
    bq = np.asarray(inputs["bq"], np.float32)
    Wconv = np.asarray(inputs["Wconv"], np.float32)
    bconv = np.asarray(inputs["bconv"], np.float32)
    gamma = np.asarray(inputs["gamma"], np.float32)
    beta = np.asarray(inputs["beta"], np.float32)
    Wkv = np.asarray(inputs["Wkv"], np.float32)
    bkv = np.asarray(inputs["bkv"], np.float32)
    Wproj = np.asarray(inputs["Wproj"], np.float32)
    bproj = np.asarray(inputs["bproj"], np.float32)

    # Xp: [ic, (m, tap)]; n = 128i + 64di + 2j + dj, m = 32i+j, tap = 2di+dj
    xp_g = []
    for b in range(B):
        xt = x[b].T.reshape(C, 32, 2, 32, 2)  # [ic, i, di, j, dj]
        xt = np.ascontiguousarray(
            xt.transpose(0, 1, 3, 2, 4).reshape(C, 8, 512)  # [ic, blk, rest]
        )
        xp_g.append(xt)

    wconvt = np.ascontiguousarray(
        Wconv.transpose(1, 2, 3, 0).reshape(C, 4, C)
    ).astype(NP_BF16)

    wkvp = gamma[:, None] * Wkv
    s_full = beta @ Wkv + bkv

    # Wproj row permutation: u' = 128 i' + 4 j + 2 di + dj -> n' = 128 i' + 64 di + 2 j + dj
    up = np.arange(C)
    i_, j_ = up // 128, (up % 128) // 4
    di, dj = (up % 4) // 2, up % 2
    nprime = 128 * i_ + 64 * di + 2 * j_ + dj
    wproj_perm = np.ascontiguousarray(Wproj[nprime, :]).astype(NP_BF16)

    eye128 = np.eye(128, dtype=np.float32).astype(NP_BF16)
    eyef = np.eye(8, dtype=np.float32)

    in_maps = []
    for core in range(N_CORES):
        b, g = divmod(core, 4)
        kcols = slice(128 * g, 128 * (g + 1))
        vcols = slice(512 + 128 * g, 512 + 128 * (g + 1))
        xp_loc = np.ascontiguousarray(xp_g[b].reshape(C, N)).astype(NP_BF16)
        small = np.zeros((128, 2), np.float32)
        small[:, 0] = bq[kcols]
        rows16 = np.zeros((8, C), np.float32)
        rows16[ROW_BCONV] = bconv
        rows16[ROW_BPROJ] = bproj
        rows16[ROW_TK, 0:128] = wkvp[:, kcols].sum(0)
        rows16[ROW_TV, 0:128] = wkvp[:, vcols].sum(0)
        rows16[ROW_SK, 0:128] = s_full[kcols]
        rows16[ROW_SV, 0:128] = s_full[vcols]
        rows16[ROW_ONES] = 1.0
        wkv2 = np.concatenate([wkvp[:, kcols], wkvp[:, vcols]], axis=1)
        in_maps.append(
            {
                "xp": xp_loc,
                "wq": np.ascontiguousarray(Wq[:, kcols]).astype(NP_BF16),
                "wconvt": wconvt,
                "wkv2": np.ascontiguousarray(wkv2).astype(NP_BF16),
                "wproj": wproj_perm,
                "small": small,
                "rows16": rows16.reshape(1, 8 * C).astype(NP_BF16),
                "eye128": eye128,
                "eyef": eyef,
            }
        )
    return in_maps


def run_spmd(inputs, **kwargs):
    """Run the SPMD kernel; returns (full_output, BassKernelResults)."""
    nc = _get_module()
    in_maps = _prep_core_inputs(inputs)
    res = run_bass_kernel_spmd(nc, in_maps, core_ids=list(range(N_CORES)), **kwargs)
    full = np.empty((B, N, C), np.float32)
    for core in range(N_CORES):
        b, g = divmod(core, 4)
        full[b, 1024 * g : 1024 * (g + 1), :] = res.results[core]["out"]
    return full, res


def kernel(**inputs) -> np.ndarray:
    full, _ = run_spmd(inputs)
    return full


# revision 24
# speedup vs baseline: 1.0588x; 1.0588x over previous
"""Trainium2 Bass kernel for PVT-style spatial-reduction multi-head attention.

Problem (hardcoded shapes, fp32 inputs):
  x [2, 4096, 512]; Wq [512,512]; Wconv [512,512,2,2] (OIHW, stride 2);
  LayerNorm over the conv's flattened spatial dim (M=1024); Wkv [1024,1024];
  attention with q [B,8,4096,64], k/v [B,8,512,64]; "faithful" reshape
  (out.transpose(0,1,3,2).reshape(B,-1,512)) before Wproj [512,512].

Sharding: 8 cores = (batch b in {0,1}) x (head-pair g in {0..3}).
Core (b,g) computes heads {2g, 2g+1} of batch b and writes output rows
[b, 1024g : 1024g+1024, :].

v3 design vs the v2 baseline (113 us):
 - x is sent host-side in a tap-expanded transposed layout Xp[ic, (m, tap)]
   (n = 128i + 64di + 2j + dj; m = 32i + j; tap = 2di + dj), so there is no
   on-chip x transpose. Q and the conv consume Xp directly; the resulting
   within-chunk column permutation of q (and of the attention output) is
   absorbed into a host-side row permutation of Wproj.
 - The stride-2 2x2 VALID conv is non-overlapping and is computed
   TRANSPOSED (xcT [m, o] = Xp^T @ Wconv'), which feeds the KV matmul with
   no transpose stage. (A 4-way m-sharded variant with a DRAM AllGather was
   tried and reverted: one collective_compute costs ~150-250 us through
   this NRT path, dwarfing the 20 us of saved conv work.)
 - LayerNorm is folded algebraically: gamma into Wkv rows (host), beta+bkv
   into a bias row s (host), so kv_noscale = xcT @ Wkv' - mu_c t + s*sqrtve_c
   with the mu/s terms as K=1 rank-1 PE matmuls into the same PSUM group.
   The per-position scale rs_c = rsqrt(var_c+eps) folds into the softmax
   exp scale (k side, per-partition AP scale) and the vaug scale (v side).
   Stats (sum x, sum x^2) come from ones-vector PE matmuls; rsqrt is
   computed as exp(-0.5*ln(v)) so the ACT engine stays on the single
   activation table holding exp/ln/copy/square/identity (no 1283ns act
   table reloads anywhere in the kernel).
 - AV is computed n-partitioned (av2[n, d] = sum_c phat[c,n]^T [v^T*rs|1]),
   so there is no AV transpose stage, and softmax denominators ride along
   as a 65th rhs column, landing pre-transposed for the normalize.
 - Engine balance: ACT does exp + PSUM->SBUF copies (one act table), DVE
   does Q epilogue, squares, AV normalize; reciprocal+normalize are
   interleaved into the chunk loop so only the projection remains as tail.
 - KV is computed in four c-column groups landing in separate kT4/vT4
   tiles (one writer each), so the first S matmul and the vaug transposes
   start as soon as their group finishes instead of waiting for the full
   512-column kT/vT accumulation.

Known dead end (do not retry naively): pairing two chunks into one
[128,1024] PSUM tile (one exp for two S^T tiles) and, separately, two exps
writing halves of one phat tile, both produced sporadic inf corruption on
one core — two WRITER instructions filling halves of a single tile race
with a reader of the second half under this Tile version's dependency
tracking. Keep one writer per tile, or add explicit ordering, if
reattempting the exp-pairing (~4us ACT win) or in-loop projection overlap
(~6us tail win).
"""

import sys

sys.path.insert(0, "/opt/trn_rl_repo")

import math

import numpy as np
import ml_dtypes

import concourse.bass as bass
import concourse.bacc as bacc
import concourse.mybir as mybir
import concourse.tile as tile
from concourse.bass_utils import run_bass_kernel_spmd

F32 = mybir.dt.float32
BF16 = mybir.dt.bfloat16
NP_BF16 = ml_dtypes.bfloat16

B, N, C = 2, 4096, 512
NH, HD, SR = 8, 64, 2
M = 1024
EPS = 1e-5
N_CORES = 8

# rows16 row map ([8, 512] bf16 host constants)
ROW_BCONV = 0
ROW_BPROJ = 1
ROW_TK = 2
ROW_TV = 3
ROW_SK = 4
ROW_SV = 5
ROW_ONES = 6


def _build_module(reps=1, bench_internal=False):
    nc = bacc.Bacc("TRN2", target_bir_lowering=False, debug=False)

    # ---- per-core DRAM tensors (data differs per core, shapes identical) ----
    KIND = "Internal" if bench_internal else "ExternalInput"
    xp = nc.dram_tensor("xp", [C, N], BF16, kind=KIND).ap()
    wq = nc.dram_tensor("wq", [C, 128], BF16, kind=KIND).ap()
    wconvt = nc.dram_tensor("wconvt", [C, 4, C], BF16, kind=KIND).ap()
    wkv2 = nc.dram_tensor("wkv2", [M, 256], BF16, kind=KIND).ap()
    wproj = nc.dram_tensor("wproj", [C, C], BF16, kind=KIND).ap()
    small = nc.dram_tensor("small", [128, 2], F32, kind="ExternalInput").ap()
    rows16 = nc.dram_tensor("rows16", [1, 8 * C], BF16, kind=KIND).ap()
    eye128 = nc.dram_tensor("eye128", [128, 128], BF16, kind=KIND).ap()
    eyef = nc.dram_tensor("eyef", [8, 8], F32, kind=KIND).ap()
    OKIND = "Internal" if bench_internal else "ExternalOutput"
    out = nc.dram_tensor("out", [2 * C, C], F32, kind=OKIND).ap()
    dummy = (
        nc.dram_tensor("bench_out", [1, 2], F32, kind="ExternalOutput").ap()
        if bench_internal
        else None
    )

    AX = mybir.AxisListType.X
    OP = mybir.AluOpType
    AF = mybir.ActivationFunctionType
    LN8 = math.log(0.125)

    with tile.TileContext(nc) as tc:
        import contextlib

        with contextlib.ExitStack() as ctx:
            persist = ctx.enter_context(tc.tile_pool(name="persist", bufs=1))
            stage = ctx.enter_context(tc.tile_pool(name="stage", bufs=3))
            ps512 = ctx.enter_context(tc.tile_pool(name="ps512", bufs=4, space="PSUM"))
            psa2 = ctx.enter_context(tc.tile_pool(name="psa2", bufs=2, space="PSUM"))
            pstp = ctx.enter_context(tc.tile_pool(name="pstp", bufs=2, space="PSUM"))

            for _rep in range(reps):
                # ---------------- weight / const loads ----------------
                wq_sb = []
                for k in range(4):
                    t = persist.tile([128, 128], BF16, name=f"wq_sb{k}", tag=f"wq{k}")
                    nc.sync.dma_start(t[:], wq[128 * k : 128 * (k + 1), :])
                    wq_sb.append(t)

                wconv_sb = []  # [ic_t] -> [128 ic, (tap 4, o 512)]
                for kt in range(4):
                    t = persist.tile([128, 2048], BF16, name=f"wconv{kt}", tag=f"wc{kt}")
                    nc.sync.dma_start(t[:], wconvt[128 * kt : 128 * (kt + 1), :, :])
                    wconv_sb.append(t)

                wkv_sb = []  # [mt] -> [128 m, 256] (cols 0:128 k, 128:256 v)
                for k in range(8):
                    t = persist.tile([128, 256], BF16, name=f"wkv_sb{k}", tag=f"wkv{k}")
                    nc.sync.dma_start(t[:], wkv2[128 * k : 128 * (k + 1), :])
                    wkv_sb.append(t)

                wproj_sb = []
                for ct in range(4):
                    t = persist.tile([128, 512], BF16, name=f"wproj{ct}", tag=f"wp{ct}")
                    nc.sync.dma_start(t[:], wproj[128 * ct : 128 * (ct + 1), :])
                    wproj_sb.append(t)

                small_sb = persist.tile([128, 2], F32, name="small_sb", tag="small")
                nc.sync.dma_start(small_sb[:], small[:, :])
                if dummy is not None and _rep == 0:
                    nc.sync.dma_start(dummy[:, :], small[0:1, 0:2])
                rows_sb = persist.tile([1, 4096], BF16, name="rows_sb", tag="rows16")
                nc.sync.dma_start(rows_sb[:], rows16[:, :])

                def crow(r, n=512):
                    return rows_sb[0:1, 512 * r : 512 * r + n]
                eye_sb = persist.tile([128, 128], BF16, name="eye_sb", tag="eye128")
                nc.sync.dma_start(eye_sb[:], eye128[:, :])
                eyef_sb = persist.tile([8, 8], F32, name="eyef_sb", tag="eyef")
                nc.sync.dma_start(eyef_sb[:], eyef[:, :])
                onec_sb = persist.tile([128, 1], BF16, name="onec_sb", tag="onec")
                nc.vector.memset(onec_sb[:], 1.0)
                ln8_sb = persist.tile([128, 1], F32, name="ln8_sb", tag="ln8")
                nc.vector.memset(ln8_sb[:], LN8)

                bq_col = small_sb[:, 0:1]

                # ---------------- x load (conv slices first) ----------------
                xp_sb = []  # [ic_t] -> [128 ic, 4096 (m,tap)]
                for kt in range(4):
                    t = persist.tile([128, N], BF16, name=f"xp_sb{kt}", tag=f"xp{kt}")
                    xp_sb.append(t)
                # three pieces, aligned to conv-block read boundaries:
                # [0:512] lets conv block 0 start after ~0.5MB of DMA;
                # the tail lands under blocks 0-3's compute.
                for lo, hi in ((0, 512), (512, 2048), (2048, 4096)):
                    for kt in range(4):
                        nc.sync.dma_start(
                            xp_sb[kt][:, lo:hi], xp[128 * kt : 128 * (kt + 1), lo:hi]
                        )
                xp4 = [t.rearrange("p (m tap) -> p m tap", m=M, tap=4) for t in xp_sb]

                # ---------------- conv (all 8 m-blocks), stats ----------------
                xcl_sb = []  # conv out [128 m, 512 o] bf16 per m-block
                sq_sb = []
                for l in range(8):
                    c_ps = ps512.tile([128, 512], F32, name="c_ps", tag="mm512")
                    first = True
                    # bconv is NOT added here: LayerNorm variance is
                    # shift-invariant and the mu*t rank-1 in KV cancels the
                    # t*bconv term exactly, so the bias-free conv output
                    # with bias-free stats yields identical kv/attention
                    # results for ANY bconv (verified to 1e-13 in numpy).
                    for kt in range(4):
                        for tap in range(4):
                            nc.tensor.matmul(
                                c_ps[:],
                                xp4[kt][:, 128 * l : 128 * (l + 1), tap],
                                wconv_sb[kt][:, 512 * tap : 512 * (tap + 1)],
                                start=first,
                                stop=(kt == 3 and tap == 3),
                            )
                            first = False
                    xcl = persist.tile([128, 512], BF16, name=f"xcl{l}", tag=f"xcl{l}")
                    nc.scalar.activation(xcl[:], c_ps[:], AF.Copy)
                    sq = persist.tile([128, 512], BF16, name=f"sq{l}", tag=f"sql{l}")
                    nc.vector.tensor_mul(sq[:], xcl[:], xcl[:])
                    xcl_sb.append(xcl)
                    sq_sb.append(sq)
                sx_ps = ps512.tile([1, 512], F32, name="sx_ps", tag="mm512")
                sq_ps = ps512.tile([1, 512], F32, name="sq_ps", tag="mm512")
                for l in range(8):
                    nc.tensor.matmul(
                        sx_ps[:], onec_sb[:], xcl_sb[l][:],
                        start=(l == 0), stop=(l == 7),
                    )
                    nc.tensor.matmul(
                        sq_ps[:], onec_sb[:], sq_sb[l][:],
                        start=(l == 0), stop=(l == 7),
                    )
                srow_sb = persist.tile([1, 1024], F32, name="srow_sb", tag="srow")
                nc.vector.tensor_copy(srow_sb[0:1, 0:512], sx_ps[:])
                nc.vector.tensor_copy(srow_sb[0:1, 512:1024], sq_ps[:])
                sx_row = srow_sb[0:1, 0:512]
                sq_row = srow_sb[0:1, 512:1024]

                # ---------------- Q projection ----------------
                qt_sb = persist.tile([128, N], BF16, name="qt_sb", tag="qt")
                for ch in range(8):
                    q_ps = ps512.tile([128, 512], F32, name="q_ps", tag="mm512")
                    for k in range(4):
                        nc.tensor.matmul(
                            q_ps[:],
                            wq_sb[k][:],
                            xp4[k][:, 128 * ch : 128 * (ch + 1), :],
                            start=(k == 0),
                            stop=(k == 3),
                        )
                    nc.vector.tensor_scalar_add(
                        qt_sb[:, 512 * ch : 512 * (ch + 1)], q_ps[:], bq_col
                    )

                # ---------------- stats math ----------------
                murow = stage.tile([1, 512], F32, name="murow", tag="murow", bufs=1)
                nc.vector.tensor_scalar_mul(murow[:], sx_row, 1.0 / M)
                negmu16 = persist.tile([1, 512], BF16, name="negmu16", tag="negmu")
                nc.vector.tensor_scalar_mul(negmu16[:], sx_row, -1.0 / M)
                verow = persist.tile([1, 512], F32, name="verow", tag="verow")
                nc.vector.tensor_scalar(
                    out=verow[:], in0=sq_row,
                    scalar1=1.0 / M, scalar2=EPS, op0=OP.mult, op1=OP.add,
                )
                mu2 = stage.tile([1, 512], F32, name="mu2", tag="mu2", bufs=1)
                nc.vector.tensor_mul(mu2[:], murow[:], murow[:])
                nc.vector.tensor_sub(verow[:], verow[:], mu2[:])
                # sqrtve row (bf16) = exp(0.5 ln ve)
                lrow = stage.tile([1, 512], F32, name="lrow", tag="lrow", bufs=1)
                nc.scalar.activation(lrow[:], verow[:], AF.Ln)
                sqve16 = persist.tile([1, 512], BF16, name="sqve16", tag="sqve")
                nc.scalar.activation(sqve16[:], lrow[:], AF.Exp, scale=0.5)
                # columns: ve -> [128, 4] via PE transpose, then exp/ln scales
                vecol_ps = ps512.tile([128, 4], F32, name="vecol_ps", tag="mm512")
                for j in range(4):
                    nc.tensor.transpose(
                        vecol_ps[:, j : j + 1],
                        verow[:, 128 * j : 128 * (j + 1)],
                        eyef_sb[0:1, 0:1],
                    )
                lcol = stage.tile([128, 4], F32, name="lcol", tag="lcol", bufs=1)
                nc.scalar.activation(lcol[:], vecol_ps[:], AF.Ln)
                esc_col = persist.tile([128, 4], F32, name="esc_col", tag="esc")
                nc.scalar.activation(esc_col[:], lcol[:], AF.Exp, scale=-0.5, bias=ln8_sb[:])
                vsc_col = persist.tile([128, 4], F32, name="vsc_col", tag="vsc")
                nc.scalar.activation(vsc_col[:], lcol[:], AF.Exp, scale=-0.5)

                # ---------------- KV ----------------
                # split by c-column group: each [128,128] group lands in its
                # own tile so the first S / vaug work starts as soon as its
                # group finishes (single writer per tile).
                kT4 = [
                    persist.tile([128, 128], BF16, name=f"kT4_{mt}", tag=f"kT{mt}")
                    for mt in range(4)
                ]
                vT4 = [
                    persist.tile([128, 128], BF16, name=f"vT4_{mt}", tag=f"vT{mt}")
                    for mt in range(4)
                ]
                for which, lo, t_row, s_row, dst4 in (
                    ("k", 0, ROW_TK, ROW_SK, kT4),
                    ("v", 128, ROW_TV, ROW_SV, vT4),
                ):
                    for g4 in range(4):
                        kv_ps = ps512.tile([128, 128], F32, name="kv_ps", tag="mm512")
                        for k in range(8):
                            nc.tensor.matmul(
                                kv_ps[:],
                                wkv_sb[k][:, lo : lo + 128],
                                xcl_sb[k][:, 128 * g4 : 128 * (g4 + 1)],
                                start=(k == 0),
                                stop=False,
                            )
                        nc.tensor.matmul(
                            kv_ps[:], crow(t_row, 128),
                            negmu16[0:1, 128 * g4 : 128 * (g4 + 1)],
                            start=False, stop=False,
                        )
                        nc.tensor.matmul(
                            kv_ps[:], crow(s_row, 128),
                            sqve16[0:1, 128 * g4 : 128 * (g4 + 1)],
                            start=False, stop=True,
                        )
                        nc.scalar.activation(dst4[g4][:], kv_ps[:], AF.Copy)

                # vaug[p][mt]: [128 c, 64 d] bf16, v^T with rs folded
                vaug_sb = []
                for p in range(2):
                    row = []
                    for mt in range(4):
                        t = persist.tile(
                            [128, 65], BF16, name=f"vaug{p}_{mt}", tag=f"va{p}{mt}"
                        )
                        tp = pstp.tile([128, 64], BF16, name="tpv", tag="ptp")
                        nc.tensor.transpose(
                            tp[:],
                            vT4[mt][64 * p : 64 * (p + 1), :],
                            eye_sb[64 * p : 64 * (p + 1), 64 * p : 64 * (p + 1)],
                        )
                        nc.vector.tensor_scalar_mul(
                            t[:, 0:64], tp[:], vsc_col[:, mt : mt + 1]
                        )
                        nc.vector.memset(t[:, 64:65], 1.0)
                        row.append(t)
                    vaug_sb.append(row)

                # ---------------- attention ----------------
                out2dT = []
                for p in range(2):
                    o_row = []
                    for ct in range(4):
                        t = persist.tile(
                            [128, 512], BF16, name=f"o2dT{p}_{ct}", tag=f"o2{p}{ct}"
                        )
                        o_row.append(t)
                    out2dT.append(o_row)

                def emit_s(ch, phat_all):
                    phat = {0: [], 1: []}
                    for mt in range(4):
                        for p in range(2):
                            s_ps = ps512.tile([128, 512], F32, name="s_ps", tag="mm512")
                            nc.tensor.matmul(
                                s_ps[:],
                                kT4[mt][64 * p : 64 * (p + 1), :],
                                qt_sb[64 * p : 64 * (p + 1), 512 * ch : 512 * (ch + 1)],
                                start=True,
                                stop=True,
                            )
                            ph = stage.tile(
                                [128, 512], BF16, name="phat", tag="phat", bufs=16
                            )
                            nc.scalar.activation(
                                ph[:], s_ps[:], AF.Exp, scale=esc_col[:, mt : mt + 1]
                            )
                            phat[p].append(ph)
                    phat_all[ch] = phat

                def emit_av(ch, phat_all):
                    phat = phat_all.pop(ch)
                    for p in range(2):
                        av2 = psa2.tile([128, 260], F32, name="av2", tag="psa2")
                        av3 = av2.rearrange("p (ct u) -> p ct u", ct=4, u=65)
                        for ct in range(4):
                            for mt in range(4):
                                nc.tensor.matmul(
                                    av3[:, ct, :],
                                    phat[p][mt][:, 128 * ct : 128 * (ct + 1)],
                                    vaug_sb[p][mt][:],
                                    start=(mt == 0),
                                    stop=(mt == 3),
                                )
                        rc = stage.tile([128, 4], F32, name="rc", tag="rc", bufs=4)
                        nc.vector.reciprocal(rc[:], av3[:, :, 64])
                        for ct in range(4):
                            o3 = out2dT[p][ct].rearrange(
                                "p (d s) -> p d s", d=64, s=8
                            )
                            nc.vector.tensor_scalar_mul(
                                o3[:, :, ch], av3[:, ct, 0:64], rc[:, ct : ct + 1]
                            )

                # software pipeline: S/exp of chunk ch+1 is emitted before
                # AV of chunk ch so the in-order PE never sits on an AV
                # matmul waiting for exp to drain.
                phat_all = {}
                emit_s(0, phat_all)
                for ch in range(8):
                    if ch + 1 < 8:
                        emit_s(ch + 1, phat_all)
                    emit_av(ch, phat_all)

                # ---------------- projection + output ----------------
                for p in range(2):
                    for rt in range(4):
                        pr_ps = ps512.tile([128, 512], F32, name="pr_ps", tag="mm512")
                        for ct in range(4):
                            nc.tensor.matmul(
                                pr_ps[:],
                                out2dT[p][ct][:, 128 * rt : 128 * (rt + 1)],
                                wproj_sb[ct][:],
                                start=(ct == 0),
                                stop=False,
                            )
                        nc.tensor.matmul(
                            pr_ps[:], crow(ROW_ONES, 128), crow(ROW_BPROJ),
                            start=False, stop=True,
                        )
                        of = stage.tile([128, 512], F32, name="of", tag="of", bufs=4)
                        if rt % 2 == 0:
                            nc.scalar.activation(of[:], pr_ps[:], AF.Copy)
                        else:
                            nc.vector.tensor_copy(of[:], pr_ps[:])
                        r0 = 512 * p + 128 * rt
                        nc.sync.dma_start(out[r0 : r0 + 128, :], of[:])

    nc.compile()
    return nc


_NC_CACHE = None


def _get_module():
    global _NC_CACHE
    if _NC_CACHE is None:
        _NC_CACHE = _build_module()
    return _NC_CACHE


def _prep_core_inputs(inputs):
    """Host-side sharding: layout/permute/cast weights, build 8 in_maps."""
    x = np.asarray(inputs["x"], np.float32)
    Wq = np.asarray(inputs["Wq"], np.float32)
    bq = np.asarray(inputs["bq"], np.float32)
    Wconv = np.asarray(inputs["Wconv"], np.float32)
    bconv = np.asarray(inputs["bconv"], np.float32)
    gamma = np.asarray(inputs["gamma"], np.float32)
    beta = np.asarray(inputs["beta"], np.float32)
    Wkv = np.asarray(inputs["Wkv"], np.float32)
    bkv = np.asarray(inputs["bkv"], np.float32)
    Wproj = np.asarray(inputs["Wproj"], np.float32)
    bproj = np.asarray(inputs["bproj"], np.float32)

    # Xp: [ic, (m, tap)]; n = 128i + 64di + 2j + dj, m = 32i+j, tap = 2di+dj
    xp_g = []
    for b in range(B):
        xt = x[b].T.reshape(C, 32, 2, 32, 2)  # [ic, i, di, j, dj]
        xt = np.ascontiguousarray(
            xt.transpose(0, 1, 3, 2, 4).reshape(C, 8, 512)  # [ic, blk, rest]
        )
        xp_g.append(xt)

    wconvt = np.ascontiguousarray(
        Wconv.transpose(1, 2, 3, 0).reshape(C, 4, C)
    ).astype(NP_BF16)

    wkvp = gamma[:, None] * Wkv
    s_full = beta @ Wkv + bkv

    # Wproj row permutation: u' = 128 i' + 4 j + 2 di + dj -> n' = 128 i' + 64 di + 2 j + dj
    up = np.arange(C)
    i_, j_ = up // 128, (up % 128) // 4
    di, dj = (up % 4) // 2, up % 2
    nprime = 128 * i_ + 64 * di + 2 * j_ + dj
    wproj_perm = np.ascontiguousarray(Wproj[nprime, :]).astype(NP_BF16)

    eye128 = np.eye(128, dtype=np.float32).astype(NP_BF16)
    eyef = np.eye(8, dtype=np.float32)

    in_maps = []
    for core in range(N_CORES):
        b, g = divmod(core, 4)
        kcols = slice(128 * g, 128 * (g + 1))
        vcols = slice(512 + 128 * g, 512 + 128 * (g + 1))
        xp_loc = np.ascontiguousarray(xp_g[b].reshape(C, N)).astype(NP_BF16)
        small = np.zeros((128, 2), np.float32)
        small[:, 0] = bq[kcols]
        rows16 = np.zeros((8, C), np.float32)
        rows16[ROW_BCONV] = bconv
        rows16[ROW_BPROJ] = bproj
        rows16[ROW_TK, 0:128] = wkvp[:, kcols].sum(0)
        rows16[ROW_TV, 0:128] = wkvp[:, vcols].sum(0)
        rows16[ROW_SK, 0:128] = s_full[kcols]
        rows16[ROW_SV, 0:128] = s_full[vcols]
        rows16[ROW_ONES] = 1.0
        wkv2 = np.concatenate([wkvp[:, kcols], wkvp[:, vcols]], axis=1)
        in_maps.append(
            {
                "xp": xp_loc,
                "wq": np.ascontiguousarray(Wq[:, kcols]).astype(NP_BF16),
                "wconvt": wconvt,
                "wkv2": np.ascontiguousarray(wkv2).astype(NP_BF16),
                "wproj": wproj_perm,
                "small": small,
                "rows16": rows16.reshape(1, 8 * C).astype(NP_BF16),
                "eye128": eye128,
                "eyef": eyef,
            }
        )
    return in_maps


def run_spmd(inputs, **kwargs):
    """Run the SPMD kernel; returns (full_output, BassKernelResults)."""
    nc = _get_module()
    in_maps = _prep_core_inputs(inputs)
    res = run_bass_kernel_spmd(nc, in_maps, core_ids=list(range(N_CORES)), **kwargs)
    full = np.empty((B, N, C), np.float32)
    for core in range(N_CORES):
        b, g = divmod(core, 4)
        full[b, 1024 * g : 1024 * (g + 1), :] = res.results[core]["out"]
    return full, res


def kernel(**inputs) -> np.ndarray:
    full, _ = run_spmd(inputs)
    return full
